# revision 59
# baseline (speedup 1.0000x reference)
"""BCSR GraphConv kernel for 8x Trainium2 NeuronCores.

Computes: out = segment_sum((X @ Wn)[edge_col] * edge_vals, edge_row) + X @ Ws

Strategy (destination-stationary; the dominant cost end-to-end is moving
inputs to the device, so the kernel minimizes per-call I/O bytes):
  - Nodes are sharded 8 ways (12500 rows per core); edges are partitioned by
    destination row (edge_row is sorted, so each core owns a contiguous edge
    range). Weights are replicated (f16, 32KB each).
  - Each core uploads ONLY its own feature shard as fp16 [12544, 128]
    (3.2MB); the full gather table [100352, 128] f16 is assembled on-device
    with 4 chunked HBM-to-HBM AllGathers over NeuronLink (far cheaper than
    uploading a replicated table 8x over the host link). Chunk j carries
    shard-row quarter j of every core and lands as table bucket j.
  - Edge metadata is 5 bytes/slot: an int16 gather index in [16, slots/16]
    ucode wrap order (replicated to 128 partitions on device), an f16 edge
    value, and a u8 destination row-within-tile (converted to f16 on device).
  - Edge rows are fetched with bulk dma_gather (int16 indices into 4 buckets
    of 25088 table rows); each destination tile of 128 nodes has a uniform
    per-bucket block budget (SPMD: one program for all cores). Padded slots
    re-gather the previous real row with val=0. Gather calls use 1024
    indices (the SWDGE ring cap; fastest measured) and round-robin over the
    4 SWDGE queues.
  - Per destination tile, ONE wide scaled-one-hot tensor [128, nb, 128]
    onehot[e, b, d] = (row_local[b*128+e] == d) * val[b*128+e]
    is built with two broadcast-AP tensor_tensor ops (vs nb per-block ops),
    and the segment-sum runs on the TensorEngine accumulating A^T directly
    in PSUM: AT[f, d] += sum_e G_blk[e, f] * onehot[e, d] (lhsT=G, rhs=oh).
  - The self branch's X^T is produced once per execution by a single XBAR
    DMA-transpose of the whole fp16 shard into a resident SBUF tile; the
    per-tile tail is then just: at16 = f16(AT);
    out_tile = at16^T @ Wn + XT_tile^T @ Ws accumulated in PSUM f32,
    stored to DRAM as f16 (converted to f32 on the host).
"""

import sys

if "/opt/trn_rl_repo" not in sys.path:
    sys.path.insert(0, "/opt/trn_rl_repo")

import numpy as np

import concourse.bacc as bacc
import concourse.mybir as mybir
import concourse.tile as tile

D = 128
P = 128
MAX_CALL = 1024  # SWDGE ring capacity per dma_gather

N_CORES = 8
NODES_PER_CORE = 12500
N_TILES = 98
SHARD_ROWS = N_TILES * P          # 12544 (12500 + 44 zero pad rows)
TABLE_ROWS = N_CORES * SHARD_ROWS  # 100352
BUCKET_TBL = TABLE_ROWS // 4       # 25088 table rows per int16 bucket
N_BUCKETS = 4
CHUNK = SHARD_ROWS // N_BUCKETS    # 3136 shard rows per chunked AllGather
GT = 8                             # dest tiles per gather group

F16 = mybir.dt.float16
F32 = mybir.dt.float32
I16 = mybir.dt.int16
I32 = mybir.dt.int32
U8 = mybir.dt.uint8


def plan_groups(n_tiles, gt):
    groups = []
    t = 0
    while t < n_tiles:
        groups.append((t, min(gt, n_tiles - t)))
        t += min(gt, n_tiles - t)
    return groups


def build_program(n_cores, n_tiles, nbk, rows_last, gt, repeat=1, mode="wideoh",
                  mc=MAX_CALL, cc_split=True):
    """One SPMD program for all cores.

    nbk: block budget (128-edge blocks) per (dest-tile, bucket)
    gt: dest tiles per gather group
    repeat: run the whole body this many times (benchmarking aid)
    mode: "wideoh" (default; fused wide one-hot build) | "full" (per-block
          one-hot) | "nocc" (replicated table upload, no collective) |
          "gather" (skip per-tile compute) | "nooh" | "notail" (probes)
    """
    nb = N_BUCKETS * nbk  # blocks per tile
    n_loc = (n_tiles - 1) * P + rows_last
    nblk_total = n_tiles * nb
    slots_total = nblk_total * P
    groups = plan_groups(n_tiles, gt)

    nc = bacc.Bacc(
        "TRN2", target_bir_lowering=False, debug=False, num_devices=n_cores,
        num_swdge_queues=4,
    )

    fshard = nc.dram_tensor("fshard", [SHARD_ROWS, D], F16, kind="ExternalInput")
    idx16 = nc.dram_tensor("idx16", [16, slots_total // 16], I16, kind="ExternalInput")
    val16 = nc.dram_tensor("val16", [P, nblk_total], F16, kind="ExternalInput")
    row8 = nc.dram_tensor("row8", [P, nblk_total], U8, kind="ExternalInput")
    wn = nc.dram_tensor("wn", [D, D], F16, kind="ExternalInput")
    ws = nc.dram_tensor("ws", [D, D], F16, kind="ExternalInput")
    out = nc.dram_tensor("out", [n_loc, D], F16, kind="ExternalOutput")
    if mode == "nocc":
        ftab = nc.dram_tensor("ftab", [TABLE_ROWS, D], F16, kind="ExternalInput")
        in_b = None
    else:
        in_b = nc.dram_tensor("in_b", [SHARD_ROWS, D], F16)
        ftab = nc.dram_tensor("ftab", [TABLE_ROWS, D], F16)

    with tile.TileContext(nc) as tc:
        with (
            tc.tile_pool(name="const", bufs=1) as cpool,
            tc.tile_pool(name="gather", bufs=2) as gpool,
            tc.tile_pool(name="onehot", bufs=(6 if mode == "wideoh" else 24)) as ohpool,
            tc.tile_pool(name="asb", bufs=3) as apool,
            tc.tile_pool(name="osb", bufs=3) as opool,
            tc.tile_pool(name="psA", bufs=3, space="PSUM") as psa_pool,
            tc.tile_pool(name="psO", bufs=2, space="PSUM") as pso_pool,
        ):
            idx_sb = cpool.tile([P, slots_total // 16], I16, tag="idx")
            rv16_sb = cpool.tile([P, nblk_total], F16, tag="rv16")
            row8_sb = cpool.tile([P, nblk_total], U8, tag="row8")
            rowm16_sb = cpool.tile([P, nblk_total], F16, tag="rowm16")
            wn_sb = cpool.tile([D, D], F16, tag="wn")
            ws_sb = cpool.tile([D, D], F16, tag="ws")
            iota_i = cpool.tile([P, P], I32, tag="iota_i")
            iota_h = cpool.tile([P, P], F16, tag="iota_h")
            # X^T for the self branch, built once per exec by XBAR DMA
            xt_all = cpool.tile([P, n_tiles * P], F16, tag="xt_all")

            # Assemble the full gather table on-device: shard -> bounce ->
            # 4 chunked AllGathers over NeuronLink into internal DRAM.
            # Chunk j carries shard rows [j*CHUNK, (j+1)*CHUNK) of every core
            # and lands as table bucket j, so bucket-j gathers can start as
            # soon as collective j completes (pipelined table assembly).
            if mode != "nocc":
                nc.sync.dma_start(in_b[:, :], fshard[:, :])
                ncc = N_BUCKETS if cc_split else 1
                csz, bsz = SHARD_ROWS // ncc, TABLE_ROWS // ncc
                for j in range(ncc):
                    nc.gpsimd.collective_compute(
                        "AllGather",
                        mybir.AluOpType.bypass,
                        replica_groups=[list(range(n_cores))],
                        ins=[in_b[j * csz : (j + 1) * csz, :].opt()],
                        outs=[ftab[j * bsz : (j + 1) * bsz, :].opt()],
                    )

            # Indices arrive in the 16-partition ucode wrap; replicate to
            # the 8 gpsimd lanes' partition groups on device.
            for k in range(8):
                nc.sync.dma_start(idx_sb[16 * k : 16 * (k + 1), :], idx16[:, :])
            nc.sync.dma_start(rv16_sb[:], val16[:])
            nc.sync.dma_start(row8_sb[:], row8[:])
            nc.vector.tensor_copy(rowm16_sb[:], row8_sb[:])
            nc.sync.dma_start(wn_sb[:], wn[:])
            nc.sync.dma_start(ws_sb[:], ws[:])
            nc.sync.dma_start_transpose(xt_all[:], fshard[:, :])
            nc.gpsimd.iota(iota_i[:], pattern=[[1, P]], base=0, channel_multiplier=0)
            nc.vector.tensor_copy(iota_h[:], iota_i[:])
            oh_const = None
            if mode == "nooh":
                oh_const = cpool.tile([P, P], F16, tag="ohc")
                nc.vector.tensor_copy(oh_const[:], iota_h[:])

            def emit_body():
                blk_base = 0  # global block id of current group's first block
                qn = 0
                for t0, gts in groups:
                    g_nblk = gts * nb
                    g = gpool.tile([P, g_nblk, D], F16, tag="g")
                    for k in range(N_BUCKETS):
                        if mode == "compute":
                            continue
                        b_lo = k * BUCKET_TBL
                        run_blk0 = k * gts * nbk
                        run_slots = gts * nbk * P
                        off = 0
                        while off < run_slots:
                            n_i = min(mc, run_slots - off)
                            blk0 = run_blk0 + off // P
                            s_abs = (blk_base + run_blk0) * P + off
                            nc.gpsimd.dma_gather(
                                g[:, blk0 : blk0 + n_i // P, :],
                                ftab[b_lo : b_lo + BUCKET_TBL, :],
                                idx_sb[:, s_abs // 16 : (s_abs + n_i) // 16],
                                n_i,
                                n_i,
                                D,
                                queue_num=qn % 4,
                            )
                            qn += 1
                            off += n_i

                    for ti in range(gts):
                        if mode == "gather":
                            continue
                        t = t0 + ti
                        psa = psa_pool.tile([P, D], F32, tag="psa")
                        ohb = None
                        if mode == "wideoh":
                            # one wide scaled-one-hot build for all nb blocks
                            sl = slice(t * nb, (t + 1) * nb)
                            ohb = ohpool.tile([P, nb, P], F16, tag="ohb")
                            rm_b = rowm16_sb[:, sl].rearrange(
                                "p (b a) -> p b a", a=1
                            ).to_broadcast([P, nb, P])
                            rv_b = rv16_sb[:, sl].rearrange(
                                "p (b a) -> p b a", a=1
                            ).to_broadcast([P, nb, P])
                            io_b = iota_h[:].rearrange(
                                "p (b a) -> p b a", b=1
                            ).to_broadcast([P, nb, P])
                            nc.any.tensor_tensor(
                                ohb[:], io_b, rm_b, mybir.AluOpType.is_equal
                            )
                            nc.any.tensor_tensor(
                                ohb[:], ohb[:], rv_b, mybir.AluOpType.mult
                            )
                        mm = 0
                        for k in range(N_BUCKETS):
                            for j in range(nbk):
                                rel = k * gts * nbk + ti * nbk + j
                                i = t * nb + k * nbk + j
                                if mode == "nooh":
                                    oh = oh_const[:]
                                elif mode == "wideoh":
                                    oh = ohb[:, k * nbk + j, :]
                                else:
                                    oht = ohpool.tile([P, P], F16, tag="oh")
                                    nc.any.tensor_scalar(
                                        oht[:],
                                        iota_h[:],
                                        rowm16_sb[:, i : i + 1],
                                        rv16_sb[:, i : i + 1],
                                        mybir.AluOpType.is_equal,
                                        mybir.AluOpType.mult,
                                    )
                                    oh = oht[:]
                                nc.tensor.matmul(
                                    psa[:],
                                    lhsT=g[:, rel, :],
                                    rhs=oh,
                                    start=(mm == 0),
                                    stop=(mm == nb - 1),
                                )
                                mm += 1

                        if mode == "notail":
                            continue
                        at16 = apool.tile([P, D], F16, tag="at")
                        nc.any.tensor_copy(at16[:], psa[:])

                        pso = pso_pool.tile([P, D], F32, tag="pso")
                        nc.tensor.matmul(
                            pso[:], lhsT=at16[:], rhs=wn_sb[:], start=True, stop=False
                        )
                        nc.tensor.matmul(
                            pso[:],
                            lhsT=xt_all[:, t * P : (t + 1) * P],
                            rhs=ws_sb[:],
                            start=False,
                            stop=True,
                        )

                        o_sb = opool.tile([P, D], F16, tag="o")
                        nc.any.tensor_copy(o_sb[:], pso[:])
                        rows = P if t < n_tiles - 1 else rows_last
                        nc.sync.dma_start(out[t * P : t * P + rows, :], o_sb[:rows, :])

                    blk_base += g_nblk

            if repeat > 1:
                with tc.For_i(0, repeat, 1):
                    emit_body()
            else:
                emit_body()

    nc.compile()
    return nc


def host_prep(features, edge_row, edge_col, edge_vals, n_cores, nodes_per_core, gt,
              layout="chunk"):
    n_nodes = features.shape[0]
    features = np.ascontiguousarray(np.asarray(features, dtype=np.float32))
    edge_row = np.asarray(edge_row).astype(np.int64)
    edge_col = np.asarray(edge_col).astype(np.int64)
    edge_vals = np.asarray(edge_vals, dtype=np.float32)

    n_tiles = (nodes_per_core + P - 1) // P
    rows_last = nodes_per_core - (n_tiles - 1) * P
    groups = plan_groups(n_tiles, gt)

    f16 = features.astype(np.float16)

    core_lo = np.searchsorted(edge_row, np.arange(n_cores) * nodes_per_core, "left")
    core_hi = np.searchsorted(
        edge_row, (np.arange(n_cores) + 1) * nodes_per_core, "left"
    )

    # uniform per-(tile,bucket) block budget across cores
    nbk = 1
    percore = []
    for m in range(n_cores):
        s, e = core_lo[m], core_hi[m]
        rows = (edge_row[s:e] - m * nodes_per_core).astype(np.int64)
        cols = edge_col[s:e]
        tile_of = rows >> 7
        if layout == "chunk":
            buck_of = (cols % nodes_per_core) // CHUNK  # source-row quarter
        else:
            buck_of = cols // (nodes_per_core * 2)  # contiguous node ranges
        key = tile_of * N_BUCKETS + buck_of
        cnt = np.bincount(key, minlength=n_tiles * N_BUCKETS)
        if cnt.size:
            nbk = max(nbk, int((cnt.max() + P - 1) // P))
        percore.append((rows, cols, edge_vals[s:e], key))

    nb = N_BUCKETS * nbk
    nblk_total = n_tiles * nb
    slots_total = nblk_total * P

    # static slot base of each (tile, bucket) section, following the group
    # ordering: [group][bucket][tile-in-group][block j][partition]
    sect_base = np.zeros((n_tiles, N_BUCKETS), np.int64)
    blk_base = 0
    for t0, gts in groups:
        for k in range(N_BUCKETS):
            for ti in range(gts):
                sect_base[t0 + ti, k] = (blk_base + k * gts * nbk + ti * nbk) * P
        blk_base += gts * nb

    core_maps = []
    for m in range(n_cores):
        rows, cols, vals, key = percore[m]
        order = np.argsort(key, kind="stable")
        rows_s, cols_s, vals_s, key_s = (
            rows[order], cols[order], vals[order], key[order],
        )
        starts = np.searchsorted(key_s, np.arange(n_tiles * N_BUCKETS))
        pos = np.arange(rows_s.size, dtype=np.int64) - starts[key_s]
        slot = sect_base[key_s // N_BUCKETS, key_s % N_BUCKETS] + pos

        # int16 index into the bucket's table-row range
        src_core = cols_s // nodes_per_core
        src_r = cols_s % nodes_per_core
        if layout == "chunk":
            # bucket j holds [core m][shard rows j*CHUNK..) (chunked AllGather)
            tidx = src_core * CHUNK + (src_r % CHUNK)
        else:
            # shard-major table from a single AllGather
            tidx = (src_core * SHARD_ROWS + src_r) % BUCKET_TBL
        slotvals = np.zeros(slots_total, np.int16)
        slotvals[slot] = tidx.astype(np.int16)
        mask = np.zeros(slots_total, bool)
        mask[slot] = True
        lastreal = np.maximum.accumulate(np.where(mask, np.arange(slots_total), 0))
        slotvals = slotvals[lastreal]
        idx16 = slotvals.reshape(slots_total // 16, 16).T.copy()

        # per-slot edge value (f16) and dest row-in-tile (u8);
        # columns are tile-major: column = tile*nb + bucket*nbk + block
        tile_s = key_s // N_BUCKETS
        buck_s = key_s % N_BUCKETS
        cols_tm = tile_s * nb + buck_s * nbk + pos // P
        val16m = np.zeros((P, nblk_total), np.float16)
        row8m = np.zeros((P, nblk_total), np.uint8)
        val16m[pos % P, cols_tm] = vals_s.astype(np.float16)
        row8m[pos % P, cols_tm] = (rows_s & 127).astype(np.uint8)

        fsh = np.zeros((SHARD_ROWS, D), np.float16)
        lo_n = m * nodes_per_core
        hi_n = min(lo_n + nodes_per_core, n_nodes)
        fsh[: hi_n - lo_n] = f16[lo_n:hi_n]

        core_maps.append(
            {"idx16": idx16, "val16": val16m, "row8": row8m, "fshard": fsh}
        )

    return core_maps, nbk, n_tiles, rows_last


_PROGRAM_CACHE = {}
_EXEC_CACHE = {}


def _get_program(key_args):
    if key_args not in _PROGRAM_CACHE:
        _PROGRAM_CACHE[key_args] = build_program(*key_args)
    return _PROGRAM_CACHE[key_args]


def _make_exec(nc, n_cores):
    """Persistent executor: jit/shard_map built once, donated zero output
    buffers generated on-device (no host->device zero upload)."""
    import jax
    import jax.numpy as jnp
    from jax.sharding import Mesh, NamedSharding, PartitionSpec
    from jax.experimental.shard_map import shard_map
    from concourse.bass2jax import (
        _bass_exec_p, install_neuronx_cc_hook, partition_id_tensor,
    )

    install_neuronx_cc_hook()
    partition_name = nc.partition_id_tensor.name if nc.partition_id_tensor else None
    in_names, out_names, out_avals = [], [], []
    for alloc in nc.m.functions[0].allocations:
        if not isinstance(alloc, mybir.MemoryLocationSet):
            continue
        name = alloc.memorylocations[0].name
        if alloc.kind == "ExternalInput":
            if name != partition_name:
                in_names.append(name)
        elif alloc.kind == "ExternalOutput":
            out_names.append(name)
            out_avals.append(
                jax.core.ShapedArray(
                    tuple(alloc.tensor_shape), mybir.dt.np(alloc.dtype)
                )
            )
    n_params = len(in_names)
    n_outs = len(out_avals)
    bind_names = list(in_names) + out_names + (
        [partition_name] if partition_name else []
    )
    donate = tuple(range(n_params, n_params + n_outs))

    def _body(*args):
        operands = list(args)
        if partition_name is not None:
            operands.append(partition_id_tensor())
        outs = _bass_exec_p.bind(
            *operands,
            out_avals=tuple(out_avals),
            in_names=tuple(bind_names),
            out_names=tuple(out_names),
            lowering_input_output_aliases=(),
            sim_require_finite=True,
            sim_require_nnan=True,
            nc=nc,
        )
        return tuple(outs)

    devices = jax.devices()[:n_cores]
    mesh = Mesh(np.asarray(devices), ("core",))
    sharded = jax.jit(
        shard_map(
            _body,
            mesh=mesh,
            in_specs=(PartitionSpec("core"),) * (n_params + n_outs),
            out_specs=(PartitionSpec("core"),) * n_outs,
            check_rep=False,
        ),
        donate_argnums=donate,
        keep_unused=True,
    )
    sh = NamedSharding(mesh, PartitionSpec("core"))
    gshapes = [
        (n_cores * a.shape[0], *a.shape[1:]) for a in out_avals
    ]
    gdtypes = [a.dtype for a in out_avals]

    def _dev_zeros():
        return tuple(
            jnp.zeros(s, d) for s, d in zip(gshapes, gdtypes)
        )

    try:
        zfn = jax.jit(_dev_zeros, out_shardings=(sh,) * n_outs)
        z = zfn()
        jax.block_until_ready(z)
    except Exception:
        zfn = lambda: tuple(  # noqa: E731
            jax.device_put(np.zeros(s, d), sh) for s, d in zip(gshapes, gdtypes)
        )

    def execute(in_maps):
        concat_in = [
            np.concatenate(
                [np.asarray(in_maps[c][name]) for c in range(n_cores)], axis=0
            )
            for name in in_names
        ]
        in_arrs = [jax.device_put(a, sh) for a in concat_in]
        outs = sharded(*in_arrs, *zfn())
        return [
            {
                name: np.asarray(outs[i]).reshape(n_cores, *out_avals[i].shape)[c]
                for i, name in enumerate(out_names)
            }
            for c in range(n_cores)
        ]

    return execute


def prepare(features, edge_row, edge_col, edge_vals, weight_neigh, weight_self,
            n_cores=N_CORES, gt=GT):
    """Host prep + program build; returns (nc, in_maps, n_nodes)."""
    n_nodes = int(np.asarray(features).shape[0])
    nodes_per_core = (n_nodes + n_cores - 1) // n_cores
    core_maps, nbk, n_tiles, rows_last = host_prep(
        features, edge_row, edge_col, edge_vals, n_cores, nodes_per_core, gt,
    )
    nc = _get_program((n_cores, n_tiles, nbk, rows_last, gt))
    wnp = np.ascontiguousarray(np.asarray(weight_neigh, dtype=np.float16))
    wsp = np.ascontiguousarray(np.asarray(weight_self, dtype=np.float16))
    in_maps = []
    for m in range(n_cores):
        im = {"wn": wnp, "ws": wsp}
        im.update(core_maps[m])
        in_maps.append(im)
    return nc, in_maps, n_nodes


def run(features, edge_row, edge_col, edge_vals, weight_neigh, weight_self,
        n_cores=N_CORES, gt=GT):
    nc, in_maps, n_nodes = prepare(
        features, edge_row, edge_col, edge_vals, weight_neigh, weight_self,
        n_cores, gt,
    )
    if id(nc) not in _EXEC_CACHE:
        _EXEC_CACHE[id(nc)] = _make_exec(nc, n_cores)
    res = _EXEC_CACHE[id(nc)](in_maps)
    out = np.concatenate([res[m]["out"] for m in range(n_cores)], axis=0)
    return out[:n_nodes].astype(np.float32)


def kernel(**inputs):
    return run(
        inputs["features"],
        inputs["edge_row"],
        inputs["edge_col"],
        inputs["edge_vals"],
        inputs["weight_neigh"],
        inputs["weight_self"],
    )
